# revision 43
# baseline (speedup 1.0000x reference)
"""FJSP decoder kernel for Trainium2, data-parallel over batch on 8 NeuronCores.

Factorized attention (see derivation in comments): q/k/v of the flattened
(job, machine) pair s=(j,m) split as x[s] = xj[j] + xm[m], so the joint
softmax over t=(j',m') factorizes exactly:

  exp(score[s,t]) = expE[s,j'] * expF[s,m']
  softmax_t(score) @ v . w2 = Nj/SE + Nm/SF      (per head)

with expE[(j,m),j'] = eA[j,j']*eC[m,j'], expF[(j,m),m'] = eB[j,m']*eD[m,m'].
The multi-head combine collapses through w2 = Wmhc @ Wshc, so v only enters
via uv = x @ (Wv_blocks @ w2) -- the v projection never runs on device.

Device-side layout: per head h (grp=h//4, strip g=h%4) one joint matmul with
stationary [kjT | kmT] ([32, 120]) against rhs [qjT | qmT] gives the full
[120, 120] block (rows 0:100 = A^T,C^T; rows 100:120 = B^T,D^T); one exp per
4-head group covers everything.  E-side contraction (K=100) reads the exp
tile in place via a 2-chunk AP; the F-side (K=120) uses a zero-framed rhs so
the B^T rows land in the same matmul.  All matmul operands are bf16 (4x PE
throughput vs f32); final softmax chain stays f32.

Host-side prep is layout/weight-folding only: weights pre-padded into the
32-strip head layout, activations pre-transposed, w2/uw folded.  One input
DMA, one output DMA.
"""

import math

import numpy as np
import ml_dtypes

import concourse.bass as bass
import concourse.mybir as mybir
import concourse.tile as tile
from concourse.bass_utils import run_bass_kernel_spmd

F32 = mybir.dt.float32
BF16 = mybir.dt.bfloat16
AF = mybir.ActivationFunctionType
OP = mybir.AluOpType
AX = mybir.AxisListType

D, H, QD = 128, 8, 16
B, J, M = 8, 100, 20
INV_SQ = 1.0 / math.sqrt(QD)  # 0.25
SD = math.sqrt(D)

# input column layout (all bf16, [128, NCOL])
EJ = 0                      # ejT [0:128, 0:100]
EM = 100                    # emT [0:128, 100:120]
WBLK = 120                  # 8 weight blocks of 128 cols each:
#   order: (k,j,g0) (k,m,g0) (k,j,g1) (k,m,g1) (q,j,g0) (q,m,g0) (q,j,g1) (q,m,g1)
UWJ = WBLK + 8 * 128        # 1144: uwj [0:128, 8]
UWM = UWJ + 8               # 1152: uwm [0:128, 8]
MK = UWM + 8                # 1160: mask [0:100, 20]
BC = MK + 20                # 1180: bias col [0:100, 1] = bias_c / sqrt(D)
NCOL = BC + 1               # 1181

# walrus ISA check rejects divide ALU ops on DVE; keep reciprocal+mul
USE_DIVIDE = False


# ---------------------------------------------------------------------------
# gen3 walrus accepts one sync-wait per instruction. Tile's kernel-tail
# drain accumulates one wait per active logical processor on a single
# Drain: spread them across engines (parallel waiting). Tile's semaphore
# pass can also attach >1 wait to ordinary instructions: shed extras onto
# same-engine NoOps inserted right before the offender.
_PATCHED = False


def _install_drain_patch():
    global _PATCHED
    if _PATCHED:
        return
    from concourse.tile import ScopedClock, TileContext

    def _split_drain_and_barrier(self, tick_clock, wait_clock):
        drain_inst = self.nc.sync.drain()
        wait_clock.add_sem_waits(
            drain_inst.ins, ScopedClock({None: tick_clock.global_clock})
        )
        si = drain_inst.ins.sync_info
        waits = list(si.on_wait) if si is not None else []
        if len(waits) > 1:
            assert not si.on_update
            sems = {s.name: s for s in self.sems.allocated().values()}
            drain_inst.ins.sync_info = None
            drain_inst.wait_op(sems[waits[0].ant_name], waits[0].wait_value, "sem-ge")
            engines = [
                self.nc.scalar,
                self.nc.vector,
                self.nc.tensor,
                self.nc.gpsimd,
                self.nc.sync,
            ]
            for i, w in enumerate(waits[1:]):
                extra = engines[i % len(engines)].drain()
                extra.wait_op(sems[w.ant_name], w.wait_value, "sem-ge")
        self.nc.all_engine_barrier()
        assert self.sems is not None
        popped = self.nc._tile_sem_poison_stack.pop()
        assert popped is self._sem_poison
        self.nc.clear_and_free_semaphores(list(self.sems.allocated().values()))

    TileContext._drain_and_barrier = _split_drain_and_barrier
    _PATCHED = True


def _split_multi_waits(nc):
    import bass_rust

    ctr = 0
    for fn in nc.m.functions:
        for bb in fn.blocks:
            il = bb.instructions
            if not any(
                i.sync_info is not None and len(i.sync_info.on_wait) > 1 for i in il
            ):
                continue
            new = []
            for ins in il:
                si = ins.sync_info
                if si is not None and len(si.on_wait) > 1:
                    waits = list(si.on_wait)
                    ups = list(si.on_update)
                    for w in waits[:-1]:
                        nop = mybir.InstNoOp(name=f"I-waitsplit-{ctr}", ins=[], outs=[])
                        ctr += 1
                        nop.engine = ins.engine
                        nop.sync_info = bass_rust.SyncInfo(on_update=[], on_wait=[w])
                        new.append(nop)
                    ins.sync_info = bass_rust.SyncInfo(
                        on_update=ups, on_wait=[waits[-1]]
                    )
                new.append(ins)
            bb.instructions = new


def _hoist_input_dma(nc):
    """Move the input DMACopy from the body block into the preamble block,
    right after the sequencer register-init moves and before the entry
    barrier. The DMA has no waits and its completion semaphore gates all
    consumers, so issuing it ~800ns earlier (in parallel with the barrier)
    is safe and shortens the critical path by the same amount."""
    fn = nc.m.functions[0]
    if len(fn.blocks) < 2:
        return
    b0, b1 = fn.blocks[0], fn.blocks[1]
    dma = None
    for ins in b1.instructions:
        if type(ins).__name__ == "InstDMACopy":
            si = ins.sync_info
            assert si is None or not si.on_wait
            dma = ins
            break
    if dma is None:
        return
    b1.instructions = [i for i in b1.instructions if i is not dma]
    pos = 1 if b0.instructions and type(b0.instructions[0]).__name__ == "InstCall" else 0
    b0.instructions = b0.instructions[:pos] + [dma] + b0.instructions[pos:]


def _ap_free_range(ap_obj):
    """[lo, hi) element range of an AP's free dims (dim 0 = partitions)."""
    lo = ap_obj.offset
    hi = lo + 1
    for stride, count in list(ap_obj.ap)[1:]:
        hi += stride * (count - 1)
    return lo, hi


def _tighten_psum_waits(nc):
    """The tile scheduler bakes each instruction's PE-tick wait from its
    scheduled slot, which over-approximates for combine ops: they end up
    waiting on unrelated later matmuls into the same (or another) PSUM
    tile. Recompute the true minimal PE tick for DVE readers of the
    mm3/mm4 tiles (cA/cB) from AP range overlap with the PE writers."""
    fn = nc.m.functions[0]
    pe_sem = None
    cnt = 0
    writers = {}  # memref -> [(lo, hi, tick)]
    for bb in fn.blocks:
        for ins in bb.instructions:
            si = ins.sync_info
            if str(ins.engine) != "EngineType.PE" or si is None:
                continue
            for u in si.on_update:
                if pe_sem is None and u.ant_name.startswith("PE"):
                    pe_sem = u.ant_name
                if u.ant_name == pe_sem:
                    cnt += u.update_value
            outs = getattr(ins, "outs", [])
            if outs:
                mr = str(getattr(outs[0], "memref", ""))
                if mr.startswith(("cA", "cB")):
                    lo, hi = _ap_free_range(outs[0])
                    writers.setdefault(mr, []).append((lo, hi, cnt))
    if pe_sem is None or not writers:
        return
    for bb in fn.blocks:
        for ins in bb.instructions:
            si = ins.sync_info
            if str(ins.engine) != "EngineType.DVE" or si is None:
                continue
            srcs = getattr(ins, "ins", [])
            if not srcs:
                continue
            mr = str(getattr(srcs[0], "memref", ""))
            if mr not in writers:
                continue
            lo, hi = _ap_free_range(srcs[0])
            need = 0
            for wlo, whi, tick in writers[mr]:
                if wlo < hi and lo < whi:
                    need = max(need, tick)
            for w in si.on_wait:
                if w.ant_name == pe_sem and w.wait_value > need > 0:
                    w.wait_value = need


def _chunk2(ap_slice, chunk_step):
    """Matmul rhs built from two equal column chunks `chunk_step` apart."""
    return bass.AP(
        tensor=ap_slice.tensor,
        offset=ap_slice.offset,
        ap=[ap_slice.ap[0], [chunk_step, 2], ap_slice.ap[1]],
    )


def _build():
    nc = bass.Bass()
    inp_d = nc.dram_tensor("inp", [D, NCOL], BF16, kind="ExternalInput")
    out_d = nc.dram_tensor("out", [J, M], F32, kind="ExternalOutput")

    with tile.TileContext(nc) as tc:
        with (
            tc.tile_pool(name="persist", bufs=1) as pp,
            tc.tile_pool(name="eero", bufs=4) as rp,
            tc.tile_pool(name="ps_proj", bufs=2, space="PSUM") as ps_proj,
            tc.tile_pool(name="ps_att", bufs=3, space="PSUM") as ps_att,
            tc.tile_pool(name="ps_out", bufs=1, space="PSUM") as ps_out,
        ):
            # ---- single input DMA, issued first ------------------------
            inp_sb = pp.tile([D, NCOL], BF16, tag="inp")
            nc.sync.dma_start(out=inp_sb, in_=inp_d[:])

            # ---- constants (no input dependency; overlap the DMA) ------
            ones = pp.tile([J, J], F32, tag="ones")
            nc.gpsimd.memset(ones, 1.0)
            # zmask: 1 on the valid m' rows 100:120, 0 on the 96:100 slack
            # (engine partition bases must be 32-aligned, so all ops on the
            # m'-rows touch the superset [96:120] and mask out 96:100)
            zmask = pp.tile([120, 1], F32, tag="zmask")
            nc.gpsimd.memset(zmask, 1.0)
            nc.gpsimd.memset(zmask[96:100, :], 0.0)
            rz = []
            for h in range(H):
                t = pp.tile([120, 40], BF16, tag=f"rz{h}")
                nc.gpsimd.memset(t[0:100, :], 0.0)
                rz.append(t)

            ejT = inp_sb[:, EJ : EJ + J]
            emT = inp_sb[:, EM : EM + M]

            # exp(mask): off the critical path, folds the mask add into the
            # final softmax as a multiply
            expmask = pp.tile([J, M], F32, tag="expmask")
            nc.scalar.activation(
                out=expmask, in_=inp_sb[0:J, MK : MK + M], func=AF.Exp, scale=1.0
            )

            # ---- projections: kq[G] = [kjT|kmT | qjT|qmT] per grp, bf16 -
            # one shared PSUM tile + one copy per grp (copies charge by
            # columns, so packing k and q halves the copy instructions)
            kt, qt = [None, None], [None, None]
            pt_list = []
            for grp in range(2):
                ps = ps_proj.tile([D, 240], F32, tag="proj")
                for half, nm in enumerate(("k", "q")):
                    blk = WBLK + (0 if nm == "k" else 4 * 128) + grp * 2 * 128
                    nc.tensor.matmul(
                        out=ps[:, 120 * half : 120 * half + J],
                        lhsT=inp_sb[:, blk : blk + D],
                        rhs=ejT,
                    )
                    nc.tensor.matmul(
                        out=ps[:, 120 * half + J : 120 * half + J + M],
                        lhsT=inp_sb[:, blk + D : blk + 2 * D],
                        rhs=emT,
                    )
                sb = pp.tile([D, 240], BF16, tag=f"kq{grp}")
                pt_list.append((nc.vector if grp == 0 else nc.scalar, sb, ps))
                kt[grp] = sb[:, 0:120]
                qt[grp] = sb[:, 120:240]

            # uv vectors: uv_ps rows 0:100 <- ej @ uwj, rows 100:120 <- em @ uwm
            uv_ps = ps_out.tile([120, 17], F32, tag="uv")
            nc.tensor.matmul(
                out=uv_ps[0:120, 0:8],
                lhsT=inp_sb[:, 0:120],
                rhs=inp_sb[:, UWJ : UWJ + 8],
            )
            nc.tensor.matmul(
                out=uv_ps[0:120, 8:16],
                lhsT=inp_sb[:, 0:120],
                rhs=inp_sb[:, UWM : UWM + 8],
            )
            for eng, sb, ps in pt_list:
                if eng is nc.scalar:
                    eng.copy(out=sb, in_=ps)
                else:
                    eng.tensor_copy(out=sb, in_=ps)
            uv_sb = pp.tile([120, 16], F32, tag="uvsb")
            nc.vector.tensor_copy(out=uv_sb, in_=uv_ps[0:120, 0:16])
            # zero the m'-side uv rows in the 96:100 slack so masked TSPs
            # reading [96:120] produce exact zeros there
            nc.vector.memset(uv_sb[96:100, 8:16], 0.0)

            # ---- attention: 4 strip-pairs (head g with head 4+g) -------
            # HW constraint: a PSUM tile must not mix different tile_position
            # values, and heads g / 4+g share tile_position (32g, 0) -- so
            # each strip-pair gets its own mm1 PSUM tile and one fused exp.
            # Per strip: ps_c[g][0:J, G, 0:80] = [SE|Nj|SF|Nm]; combine for
            # strip g runs right after its mm3/mm4 so strips 0-2 hide under
            # later strips' attention.
            ps_c = []
            for pname in ("cA", "cB"):
                ps_g = ps_out.tile([J, 4, 80], F32, tag=pname, name=pname)
                ps_c.append(ps_g)
            # ratAll[j, strip, G, E/F, m]; one fused XYZ-reduce at the end
            ratAll = pp.tile([J, 4, 2, 2, M], F32, tag="ratAll")

            for g in range(4):
                psP = ps_att.tile([120, 240], F32, tag="att")
                e1 = rp.tile([120, 280], BF16, tag="e1")
                for G in range(2):
                    nc.tensor.matmul(
                        out=psP[0:120, 120 * G : 120 * G + 120],
                        lhsT=kt[G][32 * g : 32 * g + 32, 0:120],
                        rhs=qt[G][32 * g : 32 * g + 32, 0:120],
                        tile_position=(32 * g, 0),
                    )
                nc.scalar.activation(
                    out=e1[0:120, 0:240],
                    in_=psP[0:120, 0:240],
                    func=AF.Exp,
                    scale=INV_SQ,
                )
                # F-rhs construction engine balance: Pool serializes ~123ns
                # per op, so late strips spread across DVE (66ns) and Act
                # (Copy-with-scale, fills its post-exp idle)
                for G in range(2):
                    h = 4 * G + g
                    c0 = 120 * G
                    # E-side scaled copy (in place, chunk2-addressable);
                    # strip 3's goes to Pool (free then, no ack latency) so
                    # DVE can run recipA + strip-3 F-ops back-to-back
                    jeng = nc.gpsimd if g == 3 else nc.vector
                    jeng.tensor_scalar_mul(
                        out=e1[0:J, 240 + 20 * G : 260 + 20 * G],
                        in0=e1[0:J, c0 + 100 : c0 + 120],
                        scalar1=uv_sb[0:J, h : h + 1],
                    )
                    # F-side rhs [eDT | eDT*uvm] into zero-framed rows
                    # 100:120 via the 96-aligned superset; zmask / zeroed uv
                    # rows keep the 96:100 slack exactly zero.
                    if g == 3 and G == 0:
                        nc.scalar.activation(
                            out=rz[h][96:120, 0:20],
                            in_=e1[96:120, c0 + 100 : c0 + 120],
                            func=AF.Copy, scale=zmask[96:120, 0:1],
                        )
                        nc.scalar.activation(
                            out=rz[h][96:120, 20:40],
                            in_=e1[96:120, c0 + 100 : c0 + 120],
                            func=AF.Copy, scale=uv_sb[96:120, 8 + h : 9 + h],
                        )
                    else:
                        feng = nc.vector if G == 1 else nc.gpsimd
                        feng.tensor_scalar_mul(
                            out=rz[h][96:120, 0:20],
                            in0=e1[96:120, c0 + 100 : c0 + 120],
                            scalar1=zmask[96:120, 0:1],
                        )
                        feng.tensor_scalar_mul(
                            out=rz[h][96:120, 20:40],
                            in0=e1[96:120, c0 + 100 : c0 + 120],
                            scalar1=uv_sb[96:120, 8 + h : 9 + h],
                        )
                    # E: [SE|Nj] = eAT.T @ [eCT | eCT*uvj]   (K=100)
                    nc.tensor.matmul(
                        out=ps_c[g // 2][0:J, 2 * (g % 2) + G, 0:40],
                        lhsT=e1[0:J, c0 : c0 + J],
                        rhs=_chunk2(e1[0:J, c0 + 100 : c0 + 120], 140 - 100 * G),
                    )
                    # F: [SF|Nm] = [eAT;eBT].T @ zero-framed [eDT | eDT*uvm]
                    nc.tensor.matmul(
                        out=ps_c[g // 2][0:J, 2 * (g % 2) + G, 40:80],
                        lhsT=e1[0:120, c0 : c0 + J],
                        rhs=rz[h][0:120, 0:40],
                    )
            # ---- combine: pair A fused; pair B per strip so only strip
            # 3's recip+mul sit after the last mm4. Wait-floors steer the
            # greedy FIFO scheduler: strip-3's TSPs must come first on DVE.
            t5A = ps_c[0][0:J, :, :].rearrange("p h (a b m) -> p h a b m", a=2, b=2)
            rA = pp.tile([J, 4, 2, M], F32, tag="rA")
            with tc.tile_wait_until(0.0059):
                nc.vector.reciprocal(out=rA, in_=t5A[:, :, :, 0, :])
            with tc.tile_wait_until(0.0064):
                nc.vector.scalar_tensor_tensor(
                    out=ratAll[0:J, 0:2, :, :, :], in0=t5A[:, :, :, 1, :],
                    scalar=1.0, in1=rA, op0=OP.mult, op1=OP.mult,
                )
            t5B = ps_c[1][0:J, :, :].rearrange("p h (a b m) -> p h a b m", a=2, b=2)
            rB = pp.tile([J, 4, 2, M], F32, tag="rB")
            nc.vector.reciprocal(out=rB, in_=t5B[:, :, :, 0, :])
            nc.vector.scalar_tensor_tensor(
                out=ratAll[0:J, 2:4, :, :, :], in0=t5B[:, :, :, 1, :],
                scalar=1.0, in1=rB, op0=OP.mult, op1=OP.mult,
            )

            # ---- combine tail: c1 = sum over (strip, G, E/F) -----------
            c1 = pp.tile([J, M], F32, tag="c1")
            nc.vector.reduce_sum(
                out=c1, in_=ratAll.rearrange("p s g a m -> p m s g a"),
                axis=AX.XYZ,
            )

            # ---- logits = 10*tanh((c1+bias)/sqrt(D)) + mask; softmax ---
            th = pp.tile([J, M], F32, tag="th")
            nc.scalar.activation(
                out=th,
                in_=c1,
                func=AF.Tanh,
                scale=1.0 / SD,
                bias=inp_sb[0:J, BC : BC + 1],
            )
            e10 = pp.tile([J, M], F32, tag="e10")
            nc.scalar.activation(out=e10, in_=th, func=AF.Exp, scale=10.0)
            e_sb = pp.tile([J, M], F32, tag="esb")
            s_row = pp.tile([J, 1], F32, tag="srow")
            nc.vector.scalar_tensor_tensor(
                out=e_sb,
                in0=e10,
                scalar=1.0,
                in1=expmask,
                op0=OP.mult,
                op1=OP.mult,
                accum_out=s_row,
            )
            tot_ps = uv_ps[0:J, 16:17]
            nc.tensor.matmul(out=tot_ps, lhsT=ones, rhs=s_row)
            out_t = pp.tile([J, M], F32, tag="outt")
            if USE_DIVIDE:
                nc.vector.tensor_scalar(
                    out=out_t,
                    in0=e_sb,
                    scalar1=tot_ps,
                    scalar2=None,
                    op0=OP.divide,
                )
            else:
                rtot = pp.tile([J, 1], F32, tag="rtot")
                nc.vector.reciprocal(out=rtot, in_=tot_ps)
                nc.vector.tensor_scalar_mul(out=out_t, in0=e_sb, scalar1=rtot)
            nc.sync.dma_start(out=out_d[:], in_=out_t)

    _split_multi_waits(nc)
    _hoist_input_dma(nc)
    _tighten_psum_waits(nc)
    return nc


def _pack_wblk(w):
    """[128, 64] head-major weight half -> padded 32-strip [128, 128] block."""
    blk = np.zeros((D, D), np.float32)
    for g in range(4):
        blk[:, 32 * g : 32 * g + 16] = w[:, 16 * g : 16 * g + 16]
    return blk


_NC = None
last_results = None


def kernel(**inputs):
    global _NC, last_results
    _install_drain_patch()
    if _NC is None:
        _NC = _build()

    f32 = np.float32
    Wq3 = np.asarray(inputs["Wq3"], f32)
    Wk = np.asarray(inputs["Wk"], f32)
    Wv = np.asarray(inputs["Wv"], f32)
    Wmhc = np.asarray(inputs["Wmhc"], f32)
    b_mhc = np.asarray(inputs["b_mhc"], f32).reshape(D)
    Wshc = np.asarray(inputs["Wshc"], f32).reshape(D)
    b_shc = float(np.asarray(inputs["b_shc"]).reshape(-1)[0])

    w2 = Wmhc @ Wshc  # [128]
    bias_c = float(b_mhc @ Wshc + b_shc)
    uwj = np.stack(
        [Wv[:D, 16 * h : 16 * h + 16] @ w2[16 * h : 16 * h + 16] for h in range(H)], 1
    )
    uwm = np.stack(
        [Wv[D:, 16 * h : 16 * h + 16] @ w2[16 * h : 16 * h + 16] for h in range(H)], 1
    )

    base = np.zeros((D, NCOL), f32)
    off = WBLK
    for wj, wm in ((Wk[:D], Wk[D:]), (Wq3[:D], Wq3[D:])):
        for grp in range(2):
            for w in (wj, wm):
                base[:, off : off + D] = _pack_wblk(w[:, 64 * grp : 64 * grp + 64])
                off += D
    base[:, UWJ : UWJ + 8] = uwj
    base[:, UWM : UWM + 8] = uwm
    base[0:J, BC] = bias_c / SD

    ejs = np.asarray(inputs["encoded_job"], f32)
    ems = np.asarray(inputs["encoded_machine"], f32)
    msks = np.asarray(inputs["ninf_mask"], f32)

    in_maps = []
    for b in range(B):
        ed = base.copy()
        ed[:, EJ : EJ + J] = ejs[b].T
        ed[:, EM : EM + M] = ems[b].T
        ed[0:J, MK : MK + M] = msks[b]
        in_maps.append({"inp": ed.astype(ml_dtypes.bfloat16)})

    last_results = run_bass_kernel_spmd(_NC, in_maps, core_ids=list(range(B)))
    out = np.stack(
        [last_results.results[b]["out"].reshape(J * M) for b in range(B)]
    )
    return out.astype(np.float32)
